# revision 30
# baseline (speedup 1.0000x reference)
"""Trainium2 Bass kernel for nn_KeplerDiffEq.

Computes, per orbit (4 orbits on 4 SBUF partitions):
  E = Kepler solve (Newton, seed E0 = M + e*sinM, 2 iterations, final
  trig via first-order rotation sin(E1-d) ~ sinE1 - d*cosE1)
  dr/ddr via the orbital-plane -> inertial rotation, out = [dr | ddr]  [4,6]

Accuracy (vs the 2000-step damped-Newton f32 reference, worst case over
M in [0,1)): Newton-2+rotate ~6e-4, magic-rsqrt+1NR ~3.5e-3 -> ~4e-3
total, vs the 2e-2 gate.

Engine split:
  Scalar: ONLY the three Sin ACTs (T: 12 angle lanes incl. all the
    +-pi/2-shifted copies the rotation build needs; S1, S2 per Newton
    step). Any scalar side-op risks the sim-greedy tile scheduler
    hoisting it in front of S1/S2 into the static engine order.
  Vector: the serial Newton chain + the 2-op magic-rsqrt seed (Pool
    cannot shift 32-bit ints). Order is pinned by real data deps: the
    seed F0 and EE0 are written INTO the V tile, so the rsqrt shift
    (reads V[:, 0:5]) cannot be scheduled before the Newton seed.
  GpSimd: everything independent of E (V, rotation matrix C, rsqrt
    Newton-Raphson refine, -mm^2 a^3 [x,y]), tensor_tensor only (Pool
    rejects TensorScalarPtr); constants ship as input lanes. The
    mm^2 a^3 chain writes over the dead nr tile, which creates
    write-after-read deps that sequence it behind the rsqrt refine.

Sharding: problem is tiny ("too small to shard") -> replicated SPMD on
all 8 cores; core 0's output is returned.
"""
import sys

if "/opt/trn_rl_repo" not in sys.path:
    sys.path.insert(0, "/opt/trn_rl_repo")

import numpy as np

N_ORBITS = 4
N_IN = 27
N_OUT = 6
HALF_PI = float(np.float32(np.pi / 2))
MAGIC = 0x5F3759DF

_cache = {}


def _build():
    import concourse.tile as tile
    from concourse import bacc, mybir

    AF = mybir.ActivationFunctionType
    ALU = mybir.AluOpType
    F32 = mybir.dt.float32
    I32 = mybir.dt.int32
    P = N_ORBITS

    nc = bacc.Bacc("TRN2", target_bir_lowering=False, debug=False)
    IN = nc.dram_tensor("inp", [P, N_IN], F32, kind="ExternalInput")
    OUT = nc.dram_tensor("out", [P, N_OUT], F32, kind="ExternalOutput")

    with tile.TileContext(nc) as tc:
        with tc.tile_pool(name="p", bufs=1) as pool:
            tin_t = pool.tile([P, N_IN], F32, tag="tin")
            nc.sync.dma_start(tin_t[:], IN.ap())
            tin = tin_t[:]

            m_ap = tin[:, 0:1]
            e_ap = tin[:, 12:13]
            a_ap = tin[:, 13:14]
            mm_ap = tin[:, 14:15]
            xy_ap = tin[:, 15:17]
            offs2 = tin[:, 17:19]    # [0, -pi/2]
            sgn_pm = tin[:, 19:21]   # [-1, +1]
            neg1 = tin[:, 19:20]
            nhalf = tin[:, 21:22]    # -0.5
            c1p5 = tin[:, 22:23]     # 1.5
            nxy = tin[:, 23:25]      # [-x, -y]
            c3 = tin[:, 25:26]       # 3.0

            # T = sin(angles): cols 0..11 =
            # [M, w-p2, w, W, W-p2, w, p2-w, -w, i, i-p2, p2-W, W]
            # -> [sM, n_w, s_w, s_W, n_W, s_w, c_w, -s_w, s_i, n_i, c_W, s_W]
            # (s=sin, c=cos, n=-cos; p2 = pi/2)
            T = pool.tile([P, 12], F32, tag="T")
            nc.scalar.activation(T[:], tin[:, 0:12], AF.Sin)

            # ---- GpSimd side-channel part 1 (tin/T-only deps) ----
            # V = [3a, 3a(1-e^2), x^2+y^2, F0, EE0_0, EE0_1, EE1_0, EE1_1]
            V = pool.tile([P, 8], F32, tag="V")
            nc.gpsimd.tensor_tensor(out=V[:, 0:1], in0=a_ap, in1=c3,
                                    op=ALU.mult)
            e2g = pool.tile([P, 1], F32, tag="e2g")
            nc.gpsimd.tensor_tensor(out=e2g[:], in0=e_ap, in1=e_ap,
                                    op=ALU.mult)
            tv = pool.tile([P, 1], F32, tag="tv")   # e^2 * 3a
            nc.gpsimd.tensor_tensor(out=tv[:], in0=e2g[:], in1=V[:, 0:1],
                                    op=ALU.mult)
            nc.gpsimd.tensor_tensor(out=V[:, 1:2], in0=V[:, 0:1], in1=tv[:],
                                    op=ALU.subtract)
            # -e / -a*e over the dead e2g/tv tiles: the WAR deps (tv read
            # e2g, V1 read tv) keep these AFTER V1 so the Vector rsqrt
            # seed (needs V[:,1]) is not stalled by pool FIFO order
            ne = e2g[:]
            nc.gpsimd.tensor_tensor(out=ne, in0=e_ap, in1=neg1,
                                    op=ALU.mult)
            nae = tv[:]
            nc.gpsimd.tensor_tensor(out=nae, in0=ne, in1=a_ap,
                                    op=ALU.mult)
            # rotation matrix C = [c11,c21,c31,c12,c22,c32]
            C = pool.tile([P, 6], F32, tag="C")
            Cv = C[:].rearrange("p (h j) -> p h j", h=2)[:, :, 0:2]
            nc.gpsimd.tensor_tensor(out=C[:, 2:3], in0=T[:, 2:3],
                                    in1=T[:, 8:9], op=ALU.mult)  # c31=sw*si
            nc.gpsimd.tensor_tensor(out=C[:, 5:6], in0=T[:, 6:7],
                                    in1=T[:, 8:9], op=ALU.mult)  # c32=cw*si
            z2 = pool.tile([P, 2], F32, tag="z2")   # [-sw*ci, -cw*ci]
            nc.gpsimd.tensor_tensor(out=z2[:], in0=T[:, 5:7],
                                    in1=T[:, 9:10].broadcast_to([P, 2]),
                                    op=ALU.mult)
            C4m = pool.tile([P, 2, 2], F32, tag="C4m")  # [cw,-sw](x)[cW,sW]
            nc.gpsimd.tensor_tensor(
                out=C4m[:], in0=T[:, 6:8].unsqueeze(2).broadcast_to([P, 2, 2]),
                in1=T[:, 10:12].unsqueeze(1).broadcast_to([P, 2, 2]),
                op=ALU.mult)
            Cb4 = pool.tile([P, 2, 2], F32, tag="Cb4")  # z2 (x) [sW,nW]
            nc.gpsimd.tensor_tensor(
                out=Cb4[:], in0=z2[:].unsqueeze(2).broadcast_to([P, 2, 2]),
                in1=T[:, 3:5].unsqueeze(1).broadcast_to([P, 2, 2]),
                op=ALU.mult)
            nc.gpsimd.tensor_tensor(out=Cv, in0=C4m[:], in1=Cb4[:],
                                    op=ALU.add)

            # ---- Vector: idle-start fillers, then the Newton chain ----
            sq2 = pool.tile([P, 2], F32, tag="sq2")
            nc.vector.tensor_tensor(out=sq2[:], in0=xy_ap, in1=xy_ap,
                                    op=ALU.mult)
            nc.vector.tensor_tensor(out=V[:, 2:3], in0=sq2[:, 0:1],
                                    in1=sq2[:, 1:2], op=ALU.add)
            # seed: F0 = e*sinM -> V[:,3]; EE0 = [F0, F0-pi/2] -> V[:,4:6]
            F0 = V[:, 3:4]
            nc.vector.tensor_scalar(out=F0, in0=T[:, 0:1], scalar1=e_ap,
                                    scalar2=None, op0=ALU.mult)
            EE0 = V[:, 4:6]
            nc.vector.tensor_tensor(out=EE0, in0=F0.broadcast_to([P, 2]),
                                    in1=offs2, op=ALU.add)
            S1 = pool.tile([P, 2], F32, tag="S1")
            nc.scalar.activation(S1[:], EE0, AF.Sin, bias=m_ap)

            # ---- Newton iteration 1 (Vector + Scalar ACT) ----
            f1 = pool.tile([P, 1], F32, tag="f1")
            nc.vector.tensor_scalar(out=f1[:], in0=S1[:, 0:1], scalar1=ne,
                                    scalar2=F0, op0=ALU.mult, op1=ALU.add)
            d1 = pool.tile([P, 1], F32, tag="d1")
            nc.vector.tensor_scalar(out=d1[:], in0=S1[:, 1:2], scalar1=e_ap,
                                    scalar2=1.0, op0=ALU.mult, op1=ALU.add)
            r1 = pool.tile([P, 1], F32, tag="r1")
            nc.vector.reciprocal(r1[:], d1[:])
            dF1 = pool.tile([P, 1], F32, tag="dF1")
            nc.vector.tensor_tensor(out=dF1[:], in0=f1[:], in1=r1[:],
                                    op=ALU.mult)
            EE1 = V[:, 6:8]
            nc.vector.tensor_tensor(out=EE1, in0=EE0,
                                    in1=dF1[:].broadcast_to([P, 2]),
                                    op=ALU.subtract)
            S2 = pool.tile([P, 2], F32, tag="S2")
            nc.scalar.activation(S2[:], EE1, AF.Sin, bias=m_ap)
            # magic-rsqrt seed over V[:,0:7]: the junk lanes 3..6 pin sh
            # AFTER EE1, so sh/Y run inside the S2-ACT window instead of
            # delaying f1; only lanes 0..2 are consumed downstream.
            sh = pool.tile([P, 7], I32, tag="sh")
            nc.vector.tensor_scalar(out=sh[:], in0=V[:, 0:7].bitcast(I32),
                                    scalar1=1, scalar2=None,
                                    op0=ALU.logical_shift_right)
            Y = pool.tile([P, 7], F32, tag="Y")
            nc.vector.tensor_scalar(out=Y[:].bitcast(I32), in0=sh[:],
                                    scalar1=MAGIC, scalar2=-1,
                                    op0=ALU.subtract, op1=ALU.mult)

            # ---- GpSimd: rsqrt refine on lanes 0:3 (reads Y) ----
            nr = pool.tile([P, 3], F32, tag="nr")
            nc.gpsimd.tensor_tensor(out=nr[:], in0=Y[:, 0:3], in1=Y[:, 0:3],
                                    op=ALU.mult)
            nc.gpsimd.tensor_tensor(out=nr[:], in0=nr[:], in1=V[:, 0:3],
                                    op=ALU.mult)
            nc.gpsimd.tensor_tensor(out=nr[:], in0=nr[:],
                                    in1=nhalf.broadcast_to([P, 3]),
                                    op=ALU.mult)
            nc.gpsimd.tensor_tensor(out=nr[:], in0=nr[:],
                                    in1=c1p5.broadcast_to([P, 3]),
                                    op=ALU.add)
            Y1 = pool.tile([P, 3], F32, tag="Y1")
            nc.gpsimd.tensor_tensor(out=Y1[:], in0=Y[:, 0:3], in1=nr[:],
                                    op=ALU.mult)
            # SQpm = [-sqrt(3a), +sqrt(3a(1-e^2))]
            SQ = pool.tile([P, 2], F32, tag="SQ")
            nc.gpsimd.tensor_tensor(out=SQ[:], in0=V[:, 0:2], in1=Y1[:, 0:2],
                                    op=ALU.mult)
            SQpm = pool.tile([P, 2], F32, tag="SQpm")
            nc.gpsimd.tensor_tensor(out=SQpm[:], in0=SQ[:], in1=sgn_pm,
                                    op=ALU.mult)
            # mm^2 a^3 on Scalar, data-anchored after S2 (scale=0 reads S2,
            # so the scheduler cannot hoist the chain in front of S1/S2)
            sa1 = pool.tile([P, 1], F32, tag="sa1")  # mm^2
            nc.scalar.activation(sa1[:], S2[:, 0:1], AF.Square, bias=mm_ap,
                                 scale=0.0)
            saa = pool.tile([P, 1], F32, tag="saa")  # a^2
            nc.scalar.activation(saa[:], S2[:, 0:1], AF.Square, bias=a_ap,
                                 scale=0.0)
            sa2 = pool.tile([P, 1], F32, tag="sa2")  # mm^2 a
            nc.scalar.mul(sa2[:], sa1[:], a_ap)
            sa4 = pool.tile([P, 1], F32, tag="sa4")  # mm^2 a^3
            nc.scalar.mul(sa4[:], sa2[:], saa[:])
            # w2 = -mm^2 a^3 * [x, y]  (sign via -x,-y lanes)
            w2 = pool.tile([P, 2], F32, tag="w2")
            nc.gpsimd.tensor_tensor(out=w2[:], in0=nxy,
                                    in1=sa4[:].broadcast_to([P, 2]),
                                    op=ALU.mult)

            # ---- Newton iteration 2 + first-order trig rotation ----
            f2 = pool.tile([P, 1], F32, tag="f2")
            nc.vector.scalar_tensor_tensor(out=f2[:], in0=S2[:, 0:1],
                                           scalar=ne, in1=EE1[:, 0:1],
                                           op0=ALU.mult, op1=ALU.add)
            d2 = pool.tile([P, 1], F32, tag="d2")
            nc.vector.tensor_scalar(out=d2[:], in0=S2[:, 1:2], scalar1=e_ap,
                                    scalar2=1.0, op0=ALU.mult, op1=ALU.add)
            r2 = pool.tile([P, 1], F32, tag="r2")
            nc.vector.reciprocal(r2[:], d2[:])
            dF2 = pool.tile([P, 1], F32, tag="dF2")
            nc.vector.tensor_tensor(out=dF2[:], in0=f2[:], in1=r2[:],
                                    op=ALU.mult)
            # S5 = [sin(Ef), cos(Ef)], Ef = E1 - dF2:
            #   sin ~ S2_0 + dF2*S2_1 ; cos ~ dF2*S2_0 - S2_1
            S5 = pool.tile([P, 2], F32, tag="S5")
            nc.vector.scalar_tensor_tensor(out=S5[:, 0:1], in0=S2[:, 1:2],
                                           scalar=dF2[:], in1=S2[:, 0:1],
                                           op0=ALU.mult, op1=ALU.add)
            nc.vector.scalar_tensor_tensor(out=S5[:, 1:2], in0=S2[:, 0:1],
                                           scalar=dF2[:], in1=S2[:, 1:2],
                                           op0=ALU.mult, op1=ALU.subtract)

            # ---- tail ----
            rcen = pool.tile([P, 1], F32, tag="rcen")  # a(1 - e cosEf)
            nc.vector.scalar_tensor_tensor(out=rcen[:], in0=S5[:, 1:2],
                                           scalar=nae, in1=a_ap,
                                           op0=ALU.mult, op1=ALU.add)
            rci = pool.tile([P, 1], F32, tag="rci")
            nc.vector.reciprocal(rci[:], rcen[:])
            PQ = pool.tile([P, 4], F32, tag="PQ")  # [dx, px, dy, qx]
            nc.vector.scalar_tensor_tensor(out=PQ[:, 0:4:2], in0=SQpm[:],
                                           scalar=rci[:], in1=S5[:],
                                           op0=ALU.mult, op1=ALU.mult)
            q = pool.tile([P, 1], F32, tag="q")   # rsqrt(xx+yy) * rci^2
            nc.vector.scalar_tensor_tensor(out=q[:], in0=Y1[:, 2:3],
                                           scalar=rci[:], in1=rci[:],
                                           op0=ALU.mult, op1=ALU.mult)
            nc.vector.tensor_scalar(out=PQ[:, 1:4:2], in0=w2[:],
                                    scalar1=q[:], scalar2=None,
                                    op0=ALU.mult)

            O1 = pool.tile([P, 6], F32, tag="O1")
            nc.vector.tensor_tensor(
                out=O1[:].rearrange("p (h j) -> p h j", h=2),
                in0=C[:, 0:3].unsqueeze(1).broadcast_to([P, 2, 3]),
                in1=PQ[:, 0:2].unsqueeze(2).broadcast_to([P, 2, 3]),
                op=ALU.mult)
            O2 = pool.tile([P, 6], F32, tag="O2")
            nc.vector.tensor_tensor(
                out=O2[:].rearrange("p (h j) -> p h j", h=2),
                in0=C[:, 3:6].unsqueeze(1).broadcast_to([P, 2, 3]),
                in1=PQ[:, 2:4].unsqueeze(2).broadcast_to([P, 2, 3]),
                op=ALU.mult)
            Ot = pool.tile([P, 6], F32, tag="Ot")
            nc.vector.tensor_tensor(out=Ot[:], in0=O1[:], in1=O2[:],
                                    op=ALU.add)
            nc.sync.dma_start(OUT.ap(), Ot[:])

    nc.compile()
    return nc


def _pack(a, e, i, omega, Omega, mean_motion, mean_anomaly, x):
    P = N_ORBITS
    IN = np.zeros((P, N_IN), np.float32)
    M = np.full((P,), np.float32(mean_anomaly), np.float32)
    w = np.asarray(omega, np.float32).reshape(P)
    W = np.asarray(Omega, np.float32).reshape(P)
    ii = np.asarray(i, np.float32).reshape(P)
    xf = np.asarray(x, np.float32)
    IN[:, 0] = M
    IN[:, 1] = w - HALF_PI
    IN[:, 2] = w
    IN[:, 3] = W
    IN[:, 4] = W - HALF_PI
    IN[:, 5] = w
    IN[:, 6] = HALF_PI - w
    IN[:, 7] = -w
    IN[:, 8] = ii
    IN[:, 9] = ii - HALF_PI
    IN[:, 10] = HALF_PI - W
    IN[:, 11] = W
    IN[:, 12] = np.asarray(e, np.float32).reshape(P)
    IN[:, 13] = np.asarray(a, np.float32).reshape(P)
    IN[:, 14] = np.asarray(mean_motion, np.float32).reshape(P)
    IN[:, 15] = xf[:, 0]
    IN[:, 16] = xf[:, 1]
    IN[:, 17] = 0.0
    IN[:, 18] = -HALF_PI
    IN[:, 19] = -1.0
    IN[:, 20] = 1.0
    IN[:, 21] = -0.5
    IN[:, 22] = 1.5
    IN[:, 23] = -xf[:, 0]
    IN[:, 24] = -xf[:, 1]
    IN[:, 25] = 3.0
    IN[:, 26] = M - HALF_PI
    return IN


def kernel(a, e, i, omega, Omega, mean_motion, mean_anomaly, x, _trace=False):
    from concourse.bass_utils import run_bass_kernel_spmd

    if "nc" not in _cache:
        _cache["nc"] = _build()
    nc = _cache["nc"]

    IN = _pack(a, e, i, omega, Omega, mean_motion, mean_anomaly, x)
    n_cores = 1 if _trace else 8
    res = run_bass_kernel_spmd(nc, [{"inp": IN}] * n_cores,
                               core_ids=list(range(n_cores)), trace=_trace)
    out = res.results[0]["out"].astype(np.float32)
    if _trace:
        _cache["last_result"] = res
    return out


# revision 31
# speedup vs baseline: 1.0226x; 1.0226x over previous
"""Trainium2 Bass kernel for nn_KeplerDiffEq.

Computes, per orbit (4 orbits on 4 SBUF partitions):
  E = Kepler solve (Newton, seed E0 = M + e*sinM, 2 iterations, final
  trig via first-order rotation sin(E1-d) ~ sinE1 - d*cosE1)
  dr/ddr via the orbital-plane -> inertial rotation, out = [dr | ddr]  [4,6]

Accuracy (vs the 2000-step damped-Newton f32 reference, worst case over
M in [0,1)): Newton-2+rotate ~6e-4, magic-rsqrt+1NR ~3.5e-3 -> ~4e-3
total, vs the 2e-2 gate.

Engine split:
  Scalar: ONLY the three Sin ACTs (T: 12 angle lanes incl. all the
    +-pi/2-shifted copies the rotation build needs; S1, S2 per Newton
    step). Any scalar side-op risks the sim-greedy tile scheduler
    hoisting it in front of S1/S2 into the static engine order.
  Vector: the serial Newton chain + the 2-op magic-rsqrt seed (Pool
    cannot shift 32-bit ints). Order is pinned by real data deps: the
    seed F0 and EE0 are written INTO the V tile, so the rsqrt shift
    (reads V[:, 0:5]) cannot be scheduled before the Newton seed.
  GpSimd: everything independent of E (V, rotation matrix C, rsqrt
    Newton-Raphson refine, -mm^2 a^3 [x,y]), tensor_tensor only (Pool
    rejects TensorScalarPtr); constants ship as input lanes. The
    mm^2 a^3 chain writes over the dead nr tile, which creates
    write-after-read deps that sequence it behind the rsqrt refine.

Sharding: problem is tiny ("too small to shard") -> replicated SPMD on
all 8 cores; core 0's output is returned.
"""
import sys

if "/opt/trn_rl_repo" not in sys.path:
    sys.path.insert(0, "/opt/trn_rl_repo")

import numpy as np

N_ORBITS = 4
N_IN = 27
N_OUT = 6
HALF_PI = float(np.float32(np.pi / 2))
MAGIC = 0x5F3759DF

_cache = {}


def _build():
    import concourse.tile as tile
    from concourse import bacc, mybir

    AF = mybir.ActivationFunctionType
    ALU = mybir.AluOpType
    F32 = mybir.dt.float32
    I32 = mybir.dt.int32
    P = N_ORBITS

    nc = bacc.Bacc("TRN2", target_bir_lowering=False, debug=False)
    IN = nc.dram_tensor("inp", [P, N_IN], F32, kind="ExternalInput")
    OUT = nc.dram_tensor("out", [P, N_OUT], F32, kind="ExternalOutput")

    with tile.TileContext(nc) as tc:
        with tc.tile_pool(name="p", bufs=1) as pool:
            tin_t = pool.tile([P, N_IN], F32, tag="tin")
            nc.sync.dma_start(tin_t[:], IN.ap())
            tin = tin_t[:]

            m_ap = tin[:, 0:1]
            e_ap = tin[:, 12:13]
            a_ap = tin[:, 13:14]
            mm_ap = tin[:, 14:15]
            xy_ap = tin[:, 15:17]
            offs2 = tin[:, 17:19]    # [0, -pi/2]
            sgn_pm = tin[:, 19:21]   # [-1, +1]
            neg1 = tin[:, 19:20]
            nhalf = tin[:, 21:22]    # -0.5
            c1p5 = tin[:, 22:23]     # 1.5
            nxy = tin[:, 23:25]      # [-x, -y]
            c3 = tin[:, 25:26]       # 3.0

            # T = sin(angles): cols 0..11 =
            # [M, w-p2, w, W, W-p2, w, p2-w, -w, i, i-p2, p2-W, W]
            # -> [sM, n_w, s_w, s_W, n_W, s_w, c_w, -s_w, s_i, n_i, c_W, s_W]
            # (s=sin, c=cos, n=-cos; p2 = pi/2)
            T = pool.tile([P, 12], F32, tag="T")
            nc.scalar.activation(T[:], tin[:, 0:12], AF.Sin)

            # ---- GpSimd side-channel part 1 (tin/T-only deps) ----
            # V = [3a, 3a(1-e^2), x^2+y^2, F0, EE0_0, EE0_1, EE1_0, EE1_1]
            V = pool.tile([P, 8], F32, tag="V")
            nc.gpsimd.tensor_tensor(out=V[:, 0:1], in0=a_ap, in1=c3,
                                    op=ALU.mult)
            e2g = pool.tile([P, 1], F32, tag="e2g")
            nc.gpsimd.tensor_tensor(out=e2g[:], in0=e_ap, in1=e_ap,
                                    op=ALU.mult)
            tv = pool.tile([P, 1], F32, tag="tv")   # e^2 * 3a
            nc.gpsimd.tensor_tensor(out=tv[:], in0=e2g[:], in1=V[:, 0:1],
                                    op=ALU.mult)
            nc.gpsimd.tensor_tensor(out=V[:, 1:2], in0=V[:, 0:1], in1=tv[:],
                                    op=ALU.subtract)
            # -e / -a*e over the dead e2g/tv tiles: the WAR deps (tv read
            # e2g, V1 read tv) keep these AFTER V1 so the Vector rsqrt
            # seed (needs V[:,1]) is not stalled by pool FIFO order
            ne = e2g[:]
            nc.gpsimd.tensor_tensor(out=ne, in0=e_ap, in1=neg1,
                                    op=ALU.mult)
            nae = tv[:]
            nc.gpsimd.tensor_tensor(out=nae, in0=ne, in1=a_ap,
                                    op=ALU.mult)
            # rotation matrix C = [c11,c21,c31,c12,c22,c32]
            C = pool.tile([P, 6], F32, tag="C")
            Cv = C[:].rearrange("p (h j) -> p h j", h=2)[:, :, 0:2]
            nc.gpsimd.tensor_tensor(out=C[:, 2:3], in0=T[:, 2:3],
                                    in1=T[:, 8:9], op=ALU.mult)  # c31=sw*si
            nc.gpsimd.tensor_tensor(out=C[:, 5:6], in0=T[:, 6:7],
                                    in1=T[:, 8:9], op=ALU.mult)  # c32=cw*si
            z2 = pool.tile([P, 2], F32, tag="z2")   # [-sw*ci, -cw*ci]
            nc.gpsimd.tensor_tensor(out=z2[:], in0=T[:, 5:7],
                                    in1=T[:, 9:10].broadcast_to([P, 2]),
                                    op=ALU.mult)
            C4m = pool.tile([P, 2, 2], F32, tag="C4m")  # [cw,-sw](x)[cW,sW]
            nc.gpsimd.tensor_tensor(
                out=C4m[:], in0=T[:, 6:8].unsqueeze(2).broadcast_to([P, 2, 2]),
                in1=T[:, 10:12].unsqueeze(1).broadcast_to([P, 2, 2]),
                op=ALU.mult)
            Cb4 = pool.tile([P, 2, 2], F32, tag="Cb4")  # z2 (x) [sW,nW]
            nc.gpsimd.tensor_tensor(
                out=Cb4[:], in0=z2[:].unsqueeze(2).broadcast_to([P, 2, 2]),
                in1=T[:, 3:5].unsqueeze(1).broadcast_to([P, 2, 2]),
                op=ALU.mult)
            nc.gpsimd.tensor_tensor(out=Cv, in0=C4m[:], in1=Cb4[:],
                                    op=ALU.add)

            # ---- Vector: idle-start fillers, then the Newton chain ----
            sq2 = pool.tile([P, 2], F32, tag="sq2")
            nc.vector.tensor_tensor(out=sq2[:], in0=xy_ap, in1=xy_ap,
                                    op=ALU.mult)
            nc.vector.tensor_tensor(out=V[:, 2:3], in0=sq2[:, 0:1],
                                    in1=sq2[:, 1:2], op=ALU.add)
            # seed: F0 = e*sinM -> V[:,3]; EE0 = [F0, F0-pi/2] -> V[:,4:6]
            F0 = V[:, 3:4]
            nc.vector.tensor_scalar(out=F0, in0=T[:, 0:1], scalar1=e_ap,
                                    scalar2=None, op0=ALU.mult)
            EE0 = V[:, 4:6]
            nc.vector.tensor_tensor(out=EE0, in0=F0.broadcast_to([P, 2]),
                                    in1=offs2, op=ALU.add)
            S1 = pool.tile([P, 2], F32, tag="S1")
            nc.scalar.activation(S1[:], EE0, AF.Sin, bias=m_ap)

            # ---- Newton iteration 1 (Vector + Scalar ACT) ----
            f1 = pool.tile([P, 1], F32, tag="f1")
            nc.vector.tensor_scalar(out=f1[:], in0=S1[:, 0:1], scalar1=ne,
                                    scalar2=F0, op0=ALU.mult, op1=ALU.add)
            d1 = pool.tile([P, 1], F32, tag="d1")
            nc.vector.tensor_scalar(out=d1[:], in0=S1[:, 1:2], scalar1=e_ap,
                                    scalar2=1.0, op0=ALU.mult, op1=ALU.add)
            r1 = pool.tile([P, 1], F32, tag="r1")
            nc.vector.reciprocal(r1[:], d1[:])
            dF1 = pool.tile([P, 1], F32, tag="dF1")
            nc.vector.tensor_tensor(out=dF1[:], in0=f1[:], in1=r1[:],
                                    op=ALU.mult)
            EE1 = V[:, 6:8]
            nc.vector.tensor_tensor(out=EE1, in0=EE0,
                                    in1=dF1[:].broadcast_to([P, 2]),
                                    op=ALU.subtract)
            S2 = pool.tile([P, 2], F32, tag="S2")
            nc.scalar.activation(S2[:], EE1, AF.Sin, bias=m_ap)
            # magic-rsqrt seed over V[:,0:7]: the junk lanes 3..6 pin sh
            # AFTER EE1, so sh/Y run inside the S2-ACT window instead of
            # delaying f1; only lanes 0..2 are consumed downstream.
            sh = pool.tile([P, 7], I32, tag="sh")
            nc.vector.tensor_scalar(out=sh[:], in0=V[:, 0:7].bitcast(I32),
                                    scalar1=1, scalar2=None,
                                    op0=ALU.logical_shift_right)
            Y = pool.tile([P, 7], F32, tag="Y")
            nc.vector.tensor_scalar(out=Y[:].bitcast(I32), in0=sh[:],
                                    scalar1=MAGIC, scalar2=-1,
                                    op0=ALU.subtract, op1=ALU.mult)

            # ---- GpSimd: rsqrt refine on lanes 0:3 (reads Y) ----
            nr = pool.tile([P, 3], F32, tag="nr")
            nc.gpsimd.tensor_tensor(out=nr[:], in0=Y[:, 0:3], in1=Y[:, 0:3],
                                    op=ALU.mult)
            nc.gpsimd.tensor_tensor(out=nr[:], in0=nr[:], in1=V[:, 0:3],
                                    op=ALU.mult)
            nc.gpsimd.tensor_tensor(out=nr[:], in0=nr[:],
                                    in1=nhalf.broadcast_to([P, 3]),
                                    op=ALU.mult)
            nc.gpsimd.tensor_tensor(out=nr[:], in0=nr[:],
                                    in1=c1p5.broadcast_to([P, 3]),
                                    op=ALU.add)
            Y1 = pool.tile([P, 3], F32, tag="Y1")
            nc.gpsimd.tensor_tensor(out=Y1[:], in0=Y[:, 0:3], in1=nr[:],
                                    op=ALU.mult)
            # SQpm = [-sqrt(3a), +sqrt(3a(1-e^2))]
            SQ = pool.tile([P, 2], F32, tag="SQ")
            nc.gpsimd.tensor_tensor(out=SQ[:], in0=V[:, 0:2], in1=Y1[:, 0:2],
                                    op=ALU.mult)
            SQpm = pool.tile([P, 2], F32, tag="SQpm")
            nc.gpsimd.tensor_tensor(out=SQpm[:], in0=SQ[:], in1=sgn_pm,
                                    op=ALU.mult)
            # mm^2 a^3 on Scalar, data-anchored after S2 (scale=0 reads S2,
            # so the scheduler cannot hoist the chain in front of S1/S2)
            sa1 = pool.tile([P, 1], F32, tag="sa1")  # mm^2
            nc.scalar.activation(sa1[:], S2[:, 0:1], AF.Square, bias=mm_ap,
                                 scale=0.0)
            saa = pool.tile([P, 1], F32, tag="saa")  # a^2
            nc.scalar.activation(saa[:], S2[:, 0:1], AF.Square, bias=a_ap,
                                 scale=0.0)
            sa2 = pool.tile([P, 1], F32, tag="sa2")  # mm^2 a
            nc.scalar.mul(sa2[:], sa1[:], a_ap)
            sa4 = pool.tile([P, 1], F32, tag="sa4")  # mm^2 a^3
            nc.scalar.mul(sa4[:], sa2[:], saa[:])
            # w2 = -mm^2 a^3 * [x, y]  (sign via -x,-y lanes), written
            # over the dead nr tile: the WAR dep (Y1 read nr) keeps w2
            # behind the rsqrt refine in the pool order, so SQpm is not
            # delayed past the tail's PQd.
            w2 = nr[:, 0:2]
            nc.gpsimd.tensor_tensor(out=w2, in0=nxy,
                                    in1=sa4[:].broadcast_to([P, 2]),
                                    op=ALU.mult)

            # ---- Newton iteration 2 + first-order trig rotation ----
            f2 = pool.tile([P, 1], F32, tag="f2")
            nc.vector.scalar_tensor_tensor(out=f2[:], in0=S2[:, 0:1],
                                           scalar=ne, in1=EE1[:, 0:1],
                                           op0=ALU.mult, op1=ALU.add)
            d2 = pool.tile([P, 1], F32, tag="d2")
            nc.vector.tensor_scalar(out=d2[:], in0=S2[:, 1:2], scalar1=e_ap,
                                    scalar2=1.0, op0=ALU.mult, op1=ALU.add)
            r2 = pool.tile([P, 1], F32, tag="r2")
            nc.vector.reciprocal(r2[:], d2[:])
            dF2 = pool.tile([P, 1], F32, tag="dF2")
            nc.vector.tensor_tensor(out=dF2[:], in0=f2[:], in1=r2[:],
                                    op=ALU.mult)
            # S5 = [sin(Ef), cos(Ef)], Ef = E1 - dF2:
            #   sin ~ S2_0 + dF2*S2_1 ; cos ~ dF2*S2_0 - S2_1
            S5 = pool.tile([P, 2], F32, tag="S5")
            nc.vector.scalar_tensor_tensor(out=S5[:, 0:1], in0=S2[:, 1:2],
                                           scalar=dF2[:], in1=S2[:, 0:1],
                                           op0=ALU.mult, op1=ALU.add)
            nc.vector.scalar_tensor_tensor(out=S5[:, 1:2], in0=S2[:, 0:1],
                                           scalar=dF2[:], in1=S2[:, 1:2],
                                           op0=ALU.mult, op1=ALU.subtract)

            # ---- tail ----
            rcen = pool.tile([P, 1], F32, tag="rcen")  # a(1 - e cosEf)
            nc.vector.scalar_tensor_tensor(out=rcen[:], in0=S5[:, 1:2],
                                           scalar=nae, in1=a_ap,
                                           op0=ALU.mult, op1=ALU.add)
            rci = pool.tile([P, 1], F32, tag="rci")
            nc.vector.reciprocal(rci[:], rcen[:])
            PQ = pool.tile([P, 4], F32, tag="PQ")  # [dx, px, dy, qx]
            nc.vector.scalar_tensor_tensor(out=PQ[:, 0:4:2], in0=SQpm[:],
                                           scalar=rci[:], in1=S5[:],
                                           op0=ALU.mult, op1=ALU.mult)
            q = pool.tile([P, 1], F32, tag="q")   # rsqrt(xx+yy) * rci^2
            nc.vector.scalar_tensor_tensor(out=q[:], in0=Y1[:, 2:3],
                                           scalar=rci[:], in1=rci[:],
                                           op0=ALU.mult, op1=ALU.mult)
            nc.vector.tensor_scalar(out=PQ[:, 1:4:2], in0=w2,
                                    scalar1=q[:], scalar2=None,
                                    op0=ALU.mult)

            O1 = pool.tile([P, 6], F32, tag="O1")
            nc.vector.tensor_tensor(
                out=O1[:].rearrange("p (h j) -> p h j", h=2),
                in0=C[:, 0:3].unsqueeze(1).broadcast_to([P, 2, 3]),
                in1=PQ[:, 0:2].unsqueeze(2).broadcast_to([P, 2, 3]),
                op=ALU.mult)
            O2 = pool.tile([P, 6], F32, tag="O2")
            nc.vector.tensor_tensor(
                out=O2[:].rearrange("p (h j) -> p h j", h=2),
                in0=C[:, 3:6].unsqueeze(1).broadcast_to([P, 2, 3]),
                in1=PQ[:, 2:4].unsqueeze(2).broadcast_to([P, 2, 3]),
                op=ALU.mult)
            Ot = pool.tile([P, 6], F32, tag="Ot")
            nc.vector.tensor_tensor(out=Ot[:], in0=O1[:], in1=O2[:],
                                    op=ALU.add)
            nc.sync.dma_start(OUT.ap(), Ot[:])

    nc.compile()
    return nc


def _pack(a, e, i, omega, Omega, mean_motion, mean_anomaly, x):
    P = N_ORBITS
    IN = np.zeros((P, N_IN), np.float32)
    M = np.full((P,), np.float32(mean_anomaly), np.float32)
    w = np.asarray(omega, np.float32).reshape(P)
    W = np.asarray(Omega, np.float32).reshape(P)
    ii = np.asarray(i, np.float32).reshape(P)
    xf = np.asarray(x, np.float32)
    IN[:, 0] = M
    IN[:, 1] = w - HALF_PI
    IN[:, 2] = w
    IN[:, 3] = W
    IN[:, 4] = W - HALF_PI
    IN[:, 5] = w
    IN[:, 6] = HALF_PI - w
    IN[:, 7] = -w
    IN[:, 8] = ii
    IN[:, 9] = ii - HALF_PI
    IN[:, 10] = HALF_PI - W
    IN[:, 11] = W
    IN[:, 12] = np.asarray(e, np.float32).reshape(P)
    IN[:, 13] = np.asarray(a, np.float32).reshape(P)
    IN[:, 14] = np.asarray(mean_motion, np.float32).reshape(P)
    IN[:, 15] = xf[:, 0]
    IN[:, 16] = xf[:, 1]
    IN[:, 17] = 0.0
    IN[:, 18] = -HALF_PI
    IN[:, 19] = -1.0
    IN[:, 20] = 1.0
    IN[:, 21] = -0.5
    IN[:, 22] = 1.5
    IN[:, 23] = -xf[:, 0]
    IN[:, 24] = -xf[:, 1]
    IN[:, 25] = 3.0
    IN[:, 26] = M - HALF_PI
    return IN


def kernel(a, e, i, omega, Omega, mean_motion, mean_anomaly, x, _trace=False):
    from concourse.bass_utils import run_bass_kernel_spmd

    if "nc" not in _cache:
        _cache["nc"] = _build()
    nc = _cache["nc"]

    IN = _pack(a, e, i, omega, Omega, mean_motion, mean_anomaly, x)
    n_cores = 1 if _trace else 8
    res = run_bass_kernel_spmd(nc, [{"inp": IN}] * n_cores,
                               core_ids=list(range(n_cores)), trace=_trace)
    out = res.results[0]["out"].astype(np.float32)
    if _trace:
        _cache["last_result"] = res
    return out
